# revision 6
# baseline (speedup 1.0000x reference)
import numpy as np
from contextlib import ExitStack

import concourse.bass as bass
import concourse.tile as tile
from concourse import bacc, mybir
from concourse.bass_utils import run_bass_kernel_spmd

F32 = mybir.dt.float32
F32R = mybir.dt.float32r
BF16 = mybir.dt.bfloat16
AF = mybir.ActivationFunctionType
ALU = mybir.AluOpType

NCORES = 8
B = 512
S = B // NCORES          # 64 samples per core
H = 64
IN_CH = 8
AUX = 64
CCH = [16, 32, 32]
PREV = [8, 16, 32]
FMN_N = 256
PSIZE = [PREV[i] * CCH[i] * 9 + CCH[i] for i in range(3)]   # 1168, 4640, 9248
WSIZE = [PREV[i] * CCH[i] * 9 for i in range(3)]

SELU_SCALE = 1.0507009873554805
SELU_ALPHA = 1.6732632423543772
SA = SELU_SCALE * SELU_ALPHA

# spatial geometry per layer: (Hin, pooled?, Hout)
HIN = [64, 32, 16]
HOUT = [32, 16, 16]
HPAD = [66, 34, 18]
# conv output chunking: rows per chunk so chunk <= 512 cols
CHROWS = [8, 16, 16]
NCHUNK = [8, 2, 1]


def selu_chain(nc, pool, x_ap, width, dst_ap, dst_extra=None, bias=None, biasS=None):
    """Emit SELU on x_ap [P, width] (fp32, sbuf or psum); write to dst_ap.
    bias/biasS: optional [P,1] APs (raw and pre-scaled selu'd conv bias).
    selu(x+b) = SA*exp(min(x+b,0)) - SA + SCALE*relu(x+b)."""
    m = pool.tile([128, width], F32, tag="selu_m", name="selu_m")
    if bias is not None:
        nc.vector.tensor_scalar(m[: x_ap.shape[0]], x_ap, bias, 0.0, op0=ALU.add, op1=ALU.min)
    else:
        nc.vector.tensor_scalar(m[: x_ap.shape[0]], x_ap, 0.0, None, op0=ALU.min)
    e = pool.tile([128, width], F32, tag="selu_e", name="selu_e")
    nc.scalar.activation(e[: x_ap.shape[0]], m[: x_ap.shape[0]], AF.Exp)
    r = pool.tile([128, width], F32, tag="selu_r", name="selu_r")
    if biasS is not None:
        nc.scalar.activation(r[: x_ap.shape[0]], x_ap, AF.Relu, bias=biasS, scale=SELU_SCALE)
    else:
        nc.scalar.activation(r[: x_ap.shape[0]], x_ap, AF.Relu, scale=SELU_SCALE)
    t = pool.tile([128, width], F32, tag="selu_t", name="selu_t")
    nc.vector.scalar_tensor_tensor(t[: x_ap.shape[0]], e[: x_ap.shape[0]], SA, r[: x_ap.shape[0]], op0=ALU.mult, op1=ALU.add)
    nc.vector.tensor_scalar(dst_ap, t[: x_ap.shape[0]], -SA, None, op0=ALU.add)


def build():
    nc = bacc.Bacc("TRN2", target_bir_lowering=False, debug=False, num_devices=NCORES)
    main = nc.dram_tensor("main", [S, IN_CH, H, H], F32, kind="ExternalInput").ap()
    side = nc.dram_tensor("side", [S, AUX], F32, kind="ExternalInput").ap()
    w0 = [nc.dram_tensor(f"w0_{i}", [AUX, FMN_N], F32, kind="ExternalInput").ap() for i in range(3)]
    w1 = [nc.dram_tensor(f"w1_{i}", [FMN_N, FMN_N], F32, kind="ExternalInput").ap() for i in range(3)]
    w2 = [nc.dram_tensor(f"w2_{i}", [FMN_N, PSIZE[i]], F32, kind="ExternalInput").ap() for i in range(3)]
    out = nc.dram_tensor("out", [S, CCH[2], HOUT[2], HOUT[2]], F32, kind="ExternalOutput").ap()

    # column permutation APs (rhs free AP dims) for the wb GEMMs, per layer.
    # sigma_l(s) = g_l(s)*16 + sl_l(s); rhs column j holds sample sigma^-1(j).
    # L0: s = j ; L1: j=(g,sl): s = 4*sl+g ; L2: j=(g, a,b (sl=4a+b)): s=16a+4g+b
    PERM = [
        [[1, 64]],
        [[1, 4], [4, 16]],
        [[4, 4], [16, 4], [1, 4]],
    ]

    with tile.TileContext(nc) as tc, ExitStack() as ctx:
        # ---------------- persistent tensors ----------------
        pers = ctx.enter_context(tc.tile_pool(name="pers", bufs=1))
        wconv = [
            pers.tile([128, 9 * 16 * 16], BF16, tag="wconv0", name="wconv0"),      # [32g+ci, tap*256+co*16+sl]
            pers.tile([128, 9 * 32 * 16], BF16, tag="wconv1", name="wconv1"),     # [32g+ci, tap*512+co*16+sl]
            pers.tile([128, 9 * 32 * 16], BF16, tag="wconv2", name="wconv2"),
        ]
        biasT = [pers.tile([128, 16], F32, tag=f"bias{i}", name=f"bias{i}") for i in range(3)]   # [32gc+co, gr*4+p]
        biasS = [pers.tile([128, 16], F32, tag=f"biasS{i}", name=f"biasS{i}") for i in range(3)]
        xp1 = pers.tile([128, 16 * 34 * 34], BF16, tag="xp1")
        xp2 = pers.tile([128, 16 * 18 * 18], BF16, tag="xp2")
        # borders to -1.0 once (bf16)
        for xp, hp, blocks in ((xp1, 34, 16), (xp2, 18, 16)):
            for b_ in range(blocks):
                base = b_ * hp * hp
                nc.gpsimd.memset(xp[:, base : base + hp], -1.0)
                nc.gpsimd.memset(xp[:, base + (hp - 1) * hp : base + hp * hp], -1.0)
                col = xp.rearrange("p (b y x) -> p b y x", b=blocks, y=hp)
                nc.gpsimd.memset(col[:, b_, :, 0:1], -1.0)
                nc.gpsimd.memset(col[:, b_, :, hp - 1 : hp], -1.0)

        # ---------------- hypernet ----------------
        hyp_ctx = ExitStack()
        hp_pool = hyp_ctx.enter_context(tc.tile_pool(name="hyp", bufs=2))
        hp_psum = hyp_ctx.enter_context(tc.tile_pool(name="hyp_ps", bufs=2, space="PSUM"))
        w2pool = hyp_ctx.enter_context(tc.tile_pool(name="w2", bufs=4))
        w8pool = hyp_ctx.enter_context(tc.tile_pool(name="w8", bufs=1))

        sideT = hp_pool.tile([64, 64], F32, tag="sideT")
        nc.sync.dma_start(sideT[:], side.rearrange("s f -> f s"))

        w8 = []      # per layer sbuf tensors holding selu'd weights, [p=(..), (tile)*64+sigma]
        w8b = []     # bias [Cout, 64]
        for i in range(3):
            cin, cout, ps = PREV[i], CCH[i], PSIZE[i]
            w0sb = hp_pool.tile([64, 256], F32, tag="w0sb")
            nc.sync.dma_start(w0sb[:], w0[i][:, :])
            w1sb = hp_pool.tile([128, 512], F32, tag="w1sb")  # [f_in half k at rows, k*256: cols t*256..]
            nc.sync.dma_start(w1sb[:, 0:256], w1[i][0:128, :])
            nc.sync.dma_start(w1sb[:, 256:512], w1[i][128:256, :])

            # h1 = selu(side @ W0): psum [128 fo, 64 s] x2
            h1 = hp_pool.tile([128, 128], F32, tag="h1")  # block t at cols t*64
            for t in range(2):
                ps1 = hp_psum.tile([128, 64], F32, tag="ps_small")
                nc.tensor.matmul(ps1[:], w0sb[:, t * 128 : (t + 1) * 128],
                                 sideT[:], start=True, stop=True)
                selu_chain(nc, hp_pool, ps1[:], 64, h1[:, t * 64 : (t + 1) * 64])
            # h2 = selu(h1 @ W1)
            h2 = hp_pool.tile([128, 128], F32, tag="h2")
            for t in range(2):
                ps2 = hp_psum.tile([128, 64], F32, tag="ps_small")
                for k in range(2):
                    nc.tensor.matmul(ps2[:], w1sb[:, k * 256 + t * 128 : k * 256 + (t + 1) * 128],
                                     h1[:, k * 64 : (k + 1) * 64],
                                     start=(k == 0), stop=(k == 1))
                selu_chain(nc, hp_pool, ps2[:], 64, h2[:, t * 64 : (t + 1) * 64])

            # wb = selu(h2 @ W2) with M-AP tiles; rhs columns permuted by sigma_l^-1
            def rhs_ap(k):
                base = h2[:, k * 64 : (k + 1) * 64]
                if i == 0:
                    return base
                v = h2.rearrange("p (t s) -> p t s", t=2)[:, k]
                if i == 1:
                    return v.rearrange("p (sl g) -> p g sl", g=4)
                return v.rearrange("p (a g b) -> p g a b", a=4, g=4)

            ntile = 9 * (cin * cout // 128)
            grp = 128 // cin                # co per M-tile
            nblk = cout // grp              # chunks of W2 cols
            w8sb = w8pool.tile([128, ntile * 64], BF16, tag=f"w8_{i}", name=f"w8_{i}")
            w8bsb = w8pool.tile([cout, 64], F32, tag=f"w8b_{i}")
            w8bssb = w8pool.tile([cout, 64], F32, tag=f"w8bs_{i}")
            w8.append(w8sb)
            w8b.append((w8bsb, w8bssb))
            for o in range(nblk):
                w2sb = [w2pool.tile([128, cin * 9 * grp], F32, tag="w2sb", name="w2sb") for _ in range(2)]
                for k in range(2):
                    nc.sync.dma_start(w2sb[k][:], w2[i][k * 128 : (k + 1) * 128, o * grp * cin * 9 : (o + 1) * grp * cin * 9])
                # taps 0..8 -> psum packed [128, 512] (8) + [128,64] (1)
                psA = hp_psum.tile([128, 512], F32, tag="ps_wA")
                psB = hp_psum.tile([128, 64], F32, tag="ps_small")
                for tap in range(9):
                    dst = psA[:, tap * 64 : (tap + 1) * 64] if tap < 8 else psB[:]
                    for k in range(2):
                        lhsT = w2sb[k].rearrange("p (co ci tp) -> p tp co ci", co=grp, ci=cin)[:, tap]
                        nc.tensor.matmul(dst, lhsT, rhs_ap(k),
                                         start=(k == 0), stop=(k == 1))
                selu_chain(nc, hp_pool, psA[:], 512, w8sb[:, o * 9 * 64 : o * 9 * 64 + 512])
                selu_chain(nc, hp_pool, psB[:], 64, w8sb[:, (o * 9 + 8) * 64 : (o * 9 + 9) * 64])
            # bias tile: W2 cols [ps-cout, ps)
            w2bsb = w2pool.tile([128, 2 * cout], F32, tag="w2bsb")
            for k in range(2):
                nc.sync.dma_start(w2bsb[:, k * cout : (k + 1) * cout], w2[i][k * 128 : (k + 1) * 128, ps - cout : ps])
            psb = hp_psum.tile([cout, 64], F32, tag="ps_small")
            for k in range(2):
                nc.tensor.matmul(psb[:], w2bsb[:, k * cout : (k + 1) * cout], rhs_ap(k),
                                 start=(k == 0), stop=(k == 1))
            selu_chain(nc, hp_pool, psb[:], 64, w8bsb[:])
            nc.vector.tensor_scalar(w8bssb[:], w8bsb[:], SELU_SCALE, None, op0=ALU.mult)

        # ---------------- weight remap DMAs ----------------
        for i in range(3):
            cin, cout = PREV[i], CCH[i]
            grp = 128 // cin
            w8sb = w8[i]
            w8v = w8sb.rearrange("p (tl s) -> p tl s", s=64)
            wcv = wconv[i].rearrange("p (tap co sl) -> p tap co sl", tap=9, sl=16)
            for g in range(4):
                for co in range(cout):
                    o, coL = co // grp, co % grp
                    src = w8v[coL * cin : (coL + 1) * cin, o * 9 : (o + 1) * 9, 16 * g : 16 * g + 16]
                    dst = wcv[32 * g : 32 * g + cin, :, co]
                    nc.sync.dma_start(dst, src)
            for gc in range(4):
                # src cols sigma=(g*16+4p+gc) -> [g,p] with offset gc
                s1 = w8b[i][0].rearrange("c (g p gc) -> c gc g p", g=4, gc=4)[:, gc]
                s2 = w8b[i][1].rearrange("c (g p gc) -> c gc g p", g=4, gc=4)[:, gc]
                d1 = biasT[i].rearrange("c (g p) -> c g p", g=4)[32 * gc : 32 * gc + CCH[i]]
                d2 = biasS[i].rearrange("c (g p) -> c g p", g=4)[32 * gc : 32 * gc + CCH[i]]
                nc.sync.dma_start(d1, s1)
                nc.sync.dma_start(d2, s2)

        # ---------------- conv layers ----------------
        hyp_ctx.close()
        xin_pool = ctx.enter_context(tc.tile_pool(name="xp0", bufs=5))
        cpsum = ctx.enter_context(tc.tile_pool(name="cpsum", bufs=6, space="PSUM"))
        cwork = ctx.enter_context(tc.tile_pool(name="cwork", bufs=2))
        outst = ctx.enter_context(tc.tile_pool(name="outst", bufs=3))

        for i in range(3):
            cin, cout = PREV[i], CCH[i]
            hin, hout, hpd = HIN[i], HOUT[i], HPAD[i]
            chrows, nchunk = CHROWS[i], NCHUNK[i]
            ncols = hin  # conv out cols per row
            pooled_rows = chrows // 2 if i < 2 else chrows
            pooled_cols = hin // 2 if i < 2 else hin
            for p in range(4):
                # ---- input tiles for this pass ----
                if i == 0:
                    raw = xin_pool.tile([128, 4096], F32, tag="raw", name="raw", bufs=2)
                    msrc = main.rearrange("(gr pp gc) c h w -> pp gr gc c (h w)", gr=4, pp=4)[p]
                    nc.sync.dma_start(raw[:], msrc)
                    rawb = xin_pool.tile([128, 4096], BF16, tag="rawb", name="rawb", bufs=2)
                    nc.vector.tensor_copy(rawb[:], raw[:])
                    xts = []
                    for gc in range(4):
                        xt = xin_pool.tile([128, 66 * 66], BF16, tag="xp0", name="xp0", bufs=4)
                        nc.gpsimd.memset(xt[:, 0:66], -1.0)
                        nc.gpsimd.memset(xt[:, 65 * 66 : 66 * 66], -1.0)
                        xtv = xt.rearrange("p (y x) -> p y x", y=66)
                        nc.gpsimd.memset(xtv[:, :, 0:1], -1.0)
                        nc.gpsimd.memset(xtv[:, :, 65:66], -1.0)
                        for gr in range(4):
                            nc.sync.dma_start(
                                xtv[32 * gr : 32 * gr + 8, 1:65, 1:65],
                                rawb[32 * gr + 8 * gc : 32 * gr + 8 * gc + 8].rearrange("q (h w) -> q h w", h=64),
                            )
                        xts.append(xt)

                def rhs_ap_conv(gr, gc, tap, c):
                    dy, dx = tap // 3, tap % 3
                    if i == 0:
                        v = xts[gc].rearrange("p (y x) -> p y x", y=66)
                        return v[32 * gr : 32 * gr + cin, chrows * c + dy : chrows * c + dy + chrows, dx : dx + ncols]
                    src = xp1 if i == 1 else xp2
                    blk = 4 * p + gc
                    v = src.rearrange("p (b y x) -> p b y x", b=16, y=hpd)
                    return v[32 * gr : 32 * gr + cin, blk, chrows * c + dy : chrows * c + dy + chrows, dx : dx + ncols]

                for c in range(nchunk):
                    bands = [cpsum.tile([128, chrows * ncols], F32, tag="cband", name="cband") for _ in range(4)]
                    for tap in range(9):
                        for gr in range(4):
                            for gc in range(4):
                                sl = 4 * p + gc
                                lhsT = wconv[i].rearrange("q (tap co sl) -> q tap co sl", tap=9, sl=16)[
                                    32 * gr : 32 * gr + cin, tap, :, sl]
                                rhs = rhs_ap_conv(gr, gc, tap, c)
                                nc.tensor.matmul(
                                    bands[gr][32 * gc : 32 * gc + cout, :],
                                    lhsT, rhs,
                                    start=(tap == 0), stop=(tap == 8),
                                    tile_position=(32 * gr, 32 * gc),
                                )
                    # ---- pool + bias + selu per band ----
                    for gr in range(4):
                        ba = bands[gr]
                        if i < 2:
                            pooled = cwork.tile([128, pooled_rows * pooled_cols], F32, tag="pooled")
                            vin = ba.rearrange("q (yo yi xo xi) -> q yo xo yi xi", yi=2, xi=2, yo=pooled_rows)
                            nc.vector.tensor_reduce(
                                pooled.rearrange("q (a b) -> q a b", a=pooled_rows),
                                vin, op=ALU.max, axis=mybir.AxisListType.XY)
                            xsrc = pooled[:]
                        else:
                            xsrc = ba[:]
                        width = pooled_rows * pooled_cols
                        bap = biasT[i][:, 4 * gr + p : 4 * gr + p + 1]
                        bsap = biasS[i][:, 4 * gr + p : 4 * gr + p + 1]
                        if i < 2:
                            dxp = xp1 if i == 0 else xp2
                            hpo = HPAD[i + 1]
                            blk = (4 * gr + p) if i == 0 else (4 * p + gr)
                            dv = dxp.rearrange("q (b y x) -> q b y x", b=16, y=hpo)
                            dst = dv[:, blk, 1 + pooled_rows * c : 1 + pooled_rows * (c + 1), 1 : 1 + pooled_cols]
                            selu_chain(nc, cwork, xsrc, width, dst, bias=bap, biasS=bsap)
                        else:
                            stg = outst.tile([128, 256], F32, tag="stg")
                            selu_chain(nc, cwork, xsrc, width, stg[:], bias=bap, biasS=bsap)
                            s0 = 16 * p + 4 * gr
                            nc.sync.dma_start(
                                out.rearrange("s c h w -> (s c) (h w)")[s0 * 32 : (s0 + 4) * 32, :],
                                stg[:],
                            )

    nc.compile()
    return nc


_NC_CACHE = None


def kernel(main, side, params):
    global _NC_CACHE
    if _NC_CACHE is None:
        _NC_CACHE = build()
    nc = _NC_CACHE
    main = np.asarray(main, np.float32)
    side = np.asarray(side, np.float32)
    in_maps = []
    for c in range(NCORES):
        m = {
            "main": main[c * S : (c + 1) * S],
            "side": side[c * S : (c + 1) * S],
        }
        for i in range(3):
            m[f"w0_{i}"] = np.asarray(params[i][0][0], np.float32)
            m[f"w1_{i}"] = np.asarray(params[i][1][0], np.float32)
            m[f"w2_{i}"] = np.asarray(params[i][2][0], np.float32)
        in_maps.append(m)
    import os
    trace = bool(int(os.environ.get("KERNEL_TRACE", "0")))
    res = run_bass_kernel_spmd(nc, in_maps, core_ids=list(range(NCORES)), trace=trace)
    if trace:
        print("HW exec time:", res.exec_time_ns, "ns")
        globals()["LAST_RESULT"] = res
    outs = [res.results[c]["out"] for c in range(NCORES)]
    return np.concatenate(outs, axis=0)
